# revision 14
# baseline (speedup 1.0000x reference)
"""AWQ W4A16-style quantized linear (nn_AWQLinear) on 8 Trainium2 NeuronCores.

y[m,n] = sum_k x[m,k] * ((wq[n,k]*scales[n,g(k)] + zeros[n,g(k)]) / cs[k]) + bias[n]

Column-parallel over out_features (8 cores, N_shard = 1376/core).

Fixed-point dequant via f16 exponent pinning (no 1x-rate DVE ops):

  t    = qb/16 + 16.0          ACT (free affine).  t in [16,32) -> f16
                               exponent constant, mantissa = 64h+4l is a
                               fixed-point field (exact).
  t_lo = t & 0x4C3C            DVE TS bitvec 2x  (= 16 + l/16, exact)
  wL   = t_lo * srep           DVE TT f16 2x     (= (16+l/16)*s)
  wD   = t    * srep           DVE TT f16 2x     (= (16+h+l/16)*s)
  y   += XA^T wL + XB^T wD     TensorE; XA = 16*x'_e - x'_o, XB = x'_o
                               (both host-baked into the x tiles)

XA*wL + XB*wD = s*(x_e*l + x_o*h) + 256*x_e*s.  The spurious term is
sum_g U[g,m]*s[g,n] with U[g,m] = 16*sum(XA+XB) over the group's f16
tile values -- cancelled exactly by the augmented tail matmul that also
applies zeros+bias:
  lhsT = [S | U_hi | U_lo | ones] (97 x 256), rhs = [zeros | -s | -s | bias].
Output stored f16 (budget 2e-2).

DMA: ONE fused row-contiguous ~1.3MB transfer per block-pair carrying
[qb|srep] for both blocks plus that pair's x tile (10304 B/row) -- the
whole kernel is ~17 DMAs on the nc.sync HWDGE ring, so the delivery
stream has no ordering hazards and descriptors are large.  GPSIMD stays
idle (SWDGE rings share SBUF ports with DVE).  A burst of dummy matmuls
at t=0 warms the PE HAM clock gate (1.2 -> 2.4 GHz).
"""
import numpy as np

import concourse.bacc as bacc
import concourse.mybir as mybir
from concourse import tile
from concourse.bass_utils import run_bass_kernel_spmd

IN_F = 4096          # K
OUT_F = 11008        # N
M_TOK = 256          # M
NCORES = 8
NSH = OUT_F // NCORES   # 1376
NB = IN_F // 256        # 16 blocks of 128 byte-rows (256 k-values each)
NP = NB // 2            # 8 pairs
CHUNKS = [(0, 512), (512, 512), (1024, NSH - 1024)]
BLKW = NSH + 2 * NSH    # 4128 bytes: [qb u8 | srep f16] per block row
XW = 4 * M_TOK * 2      # 2048 bytes of x per pair row (4 m-tiles f16)
PRW = 2 * BLKW + XW     # 10304 bytes per fused pair row

F32, F16, U8, U16 = (mybir.dt.float32, mybir.dt.float16, mybir.dt.uint8,
                     mybir.dt.uint16)


def _build_nc():
    nc = bacc.Bacc("TRN2", target_bir_lowering=False, debug=False,
                   num_devices=NCORES)

    blk_d = nc.dram_tensor("blk", [NP * 128, PRW], U8, kind="ExternalInput")
    s16_d = nc.dram_tensor("s16", [97, M_TOK], F16, kind="ExternalInput")
    zbT_d = nc.dram_tensor("zbT", [97, NSH], F16, kind="ExternalInput")
    y_d = nc.dram_tensor("y", [M_TOK, NSH], F16, kind="ExternalOutput")

    A = mybir.AluOpType
    W2 = 2 * NSH

    with tile.TileContext(nc) as tc:
        with (
            tc.tile_pool(name="const", bufs=1) as cpool,
            tc.tile_pool(name="blk", bufs=4) as blkpool,
            tc.tile_pool(name="t16", bufs=3) as tpool,
            tc.tile_pool(name="tl", bufs=3) as tlpool,
            tc.tile_pool(name="w", bufs=6) as wpool,
            tc.tile_pool(name="yout", bufs=3) as ypool,
            tc.tile_pool(name="ps", bufs=1, space="PSUM") as pspool,
        ):
            # ---- PE warmup: dummy matmuls to flip HAM to 8/8 early ----
            warm = cpool.tile([128, 640], F16)
            nc.vector.memset(warm[:], 0.0)
            warm_ps = pspool.tile([128, 512], F32, tag="warm_ps")
            for _ in range(8):
                nc.tensor.matmul(warm_ps[:], warm[:, 0:128], warm[:, 128:640],
                                 start=True, stop=True)

            # ---- psum accumulators ----
            y_ps = [[pspool.tile([128, w], F32, tag=f"yps_{m}_{ci}",
                                 name=f"yps_{m}_{ci}")
                     for ci, (_, w) in enumerate(CHUNKS)] for m in range(2)]

            state = {}

            def xtile(p, t):
                """m-tile t (0..3) of pair p's x: [XA_2p|XB_2p|XA_|XB_]."""
                xv = state[("blk", p)][:, 2 * BLKW:PRW].bitcast(F16)
                return xv[:, t * M_TOK:(t + 1) * M_TOK]

            def load_pair(p):
                blkp = blkpool.tile([128, PRW], U8, tag="blk",
                                    name=f"blk_{p}")
                src = blk_d[p * 128:(p + 1) * 128, :]
                if p == 0:
                    # split for head latency; x rides part b
                    nc.sync.dma_start(blkp[:, 0:BLKW], src[:, 0:BLKW])
                    nc.sync.dma_start(blkp[:, BLKW:], src[:, BLKW:])
                else:
                    nc.sync.dma_start(blkp[:], src)
                state[("blk", p)] = blkp

            def dequant(p, j=None):
                """Dequant a pair (or one block j of pair 0)."""
                blkp = state[("blk", p)]
                pairv = blkp[:, 0:2 * BLKW].rearrange("p (j w) -> p j w", j=2)
                if j is None:
                    qb = pairv[:, :, 0:NSH]
                    srep = pairv[:, :, NSH:BLKW].bitcast(F16)
                    tt = tpool.tile([128, W2], F16, tag="t16", name=f"t16_{p}")
                    tl = tlpool.tile([128, W2], F16, tag="tl", name=f"tl_{p}")
                    wL = wpool.tile([128, W2], F16, tag="wL", name=f"wL_{p}")
                    wD = wpool.tile([128, W2], F16, tag="wD", name=f"wD_{p}")
                    tv = tt[:].rearrange("p (j n) -> p j n", j=2)
                    nc.scalar.activation(tv, qb,
                                         mybir.ActivationFunctionType.Copy,
                                         bias=16.0, scale=0.0625)
                    nc.vector.tensor_scalar(tl[:].bitcast(U16),
                                            in0=tt[:].bitcast(U16),
                                            scalar1=0x4C3C, scalar2=None,
                                            op0=A.bitwise_and)
                    nc.vector.tensor_tensor(
                        wL[:].rearrange("p (j n) -> p j n", j=2),
                        tl[:].rearrange("p (j n) -> p j n", j=2), srep, A.mult)
                    nc.vector.tensor_tensor(
                        wD[:].rearrange("p (j n) -> p j n", j=2),
                        tv, srep, A.mult)
                    state[p] = (wL, wD)
                    return
                qb = pairv[:, j, 0:NSH]
                srep = pairv[:, j, NSH:BLKW].bitcast(F16)
                if j == 0:
                    tt = tpool.tile([128, W2], F16, tag="t16", name=f"t16_{p}")
                    tl = tlpool.tile([128, W2], F16, tag="tl", name=f"tl_{p}")
                    wL = wpool.tile([128, W2], F16, tag="wL", name=f"wL_{p}")
                    wD = wpool.tile([128, W2], F16, tag="wD", name=f"wD_{p}")
                    state[("tiles", p)] = (tt, tl, wL, wD)
                else:
                    tt, tl, wL, wD = state[("tiles", p)]
                sl = slice(j * NSH, (j + 1) * NSH)
                nc.scalar.activation(tt[:, sl], qb,
                                     mybir.ActivationFunctionType.Copy,
                                     bias=16.0, scale=0.0625)
                nc.vector.tensor_scalar(tl[:, sl].bitcast(U16),
                                        in0=tt[:, sl].bitcast(U16),
                                        scalar1=0x4C3C, scalar2=None,
                                        op0=A.bitwise_and)
                nc.vector.tensor_tensor(wL[:, sl], tl[:, sl], srep, A.mult)
                nc.vector.tensor_tensor(wD[:, sl], tt[:, sl], srep, A.mult)
                if j == 1:
                    state[p] = (wL, wD)

            def mms(b):
                p, j = b // 2, b % 2
                wL, wD = state[p]
                XA = xtile(p, 2 * j)      # 16*x'_e - x'_o (host-baked)
                XB = xtile(p, 2 * j + 1)  # x'_o
                if b % 2 == 1:
                    state.pop(p)
                    state.pop(("tiles", p), None)
                sl = slice(j * NSH, (j + 1) * NSH)
                wLs, wDs = wL[:, sl], wD[:, sl]
                if b < NB - 2:
                    for w, xt in ((wLs, XA), (wDs, XB)):
                        for m in range(2):
                            for ci, (c0, cw) in enumerate(CHUNKS):
                                nc.tensor.matmul(
                                    y_ps[m][ci][:],
                                    xt[:, m * 128:(m + 1) * 128],
                                    w[:, c0:c0 + cw],
                                    start=(b == 0 and w is wLs), stop=False)
                    return
                if b == NB - 2:
                    return  # emitted together with b == NB-1 below
                # last pair: chunk-major over blocks 14+15 so each chunk's
                # correction-MM + copy + store overlap later chunks' matmuls
                for ci, (c0, cw) in enumerate(CHUNKS):
                    for jj in range(2):
                        bsl = slice(jj * NSH + c0, jj * NSH + c0 + cw)
                        XAj = xtile(p, 2 * jj)
                        XBj = xtile(p, 2 * jj + 1)
                        for w, xt in ((wL, XAj), (wD, XBj)):
                            for m in range(2):
                                nc.tensor.matmul(
                                    y_ps[m][ci][:],
                                    xt[:, m * 128:(m + 1) * 128],
                                    w[:, bsl],
                                    start=False, stop=False)
                    for m in range(2):
                        nc.tensor.matmul(y_ps[m][ci][:],
                                         s16[:, m * 128:(m + 1) * 128],
                                         zbT[:, c0:c0 + cw],
                                         start=False, stop=True)
                        ysb = ypool.tile([128, cw], F16, tag=f"ysb_{ci}",
                                         name=f"ysb_{m}_{ci}")
                        if (m + ci) % 2:
                            nc.scalar.copy(ysb[:], y_ps[m][ci][:])
                        else:
                            nc.vector.tensor_copy(ysb[:], y_ps[m][ci][:])
                        nc.sync.dma_start(
                            y_d[m * 128:(m + 1) * 128, c0:c0 + cw], ysb[:])

            # ---- software-pipelined main loop (pair granularity) ----
            # JIT DMA issue: at most ~2 pair-loads in flight, so the
            # single-queue round-robin cannot delay the oldest pair's
            # completion behind younger transfers.
            load_pair(0)
            dequant(0, j=0)
            load_pair(1)
            # tail-only constants: tiny, issue while the ring is quiet
            s16 = cpool.tile([97, M_TOK], F16)
            nc.sync.dma_start(s16[:], s16_d[:])
            zbT = cpool.tile([97, NSH], F16)
            nc.sync.dma_start(zbT[:], zbT_d[:])
            dequant(0, j=1)
            for p in range(1, NP + 1):
                if p < NP:
                    if p + 1 < NP:
                        load_pair(p + 1)
                    dequant(p)
                mms(2 * (p - 1))
                mms(2 * (p - 1) + 1)

    nc.compile()
    return nc


def _host_prep(x, qweight, scales, zeros, channel_scales, bias):
    cs = np.asarray(channel_scales, np.float32)
    x2 = np.asarray(x, dtype=np.float32).reshape(M_TOK, IN_F) / cs
    qw = np.asarray(qweight)
    if qw.dtype != np.uint8:
        qw = qw.astype(np.uint8)
    qwT = np.ascontiguousarray(qw.T)                      # [K/2, N]

    # x tiles: block b byte-row q holds k_even=256b+2q, k_odd=256b+2q+1.
    # XA-slot (even) = 16*x'_e - x'_o, XB-slot (odd) = x'_o  (host-baked)
    xe = x2[:, 0::2]
    xo = x2[:, 1::2]
    xs = np.empty_like(x2)
    xs[:, 0::2] = 16.0 * xe - xo
    xs[:, 1::2] = xo
    q = np.arange(128)
    perm = np.empty(IN_F, np.int64)
    for b in range(NB):
        perm[(2 * b) * 128 + q] = 256 * b + 2 * q
        perm[(2 * b + 1) * 128 + q] = 256 * b + 2 * q + 1
    xT_perm = xs.T[perm]                                  # [K, M]
    xT_b = np.ascontiguousarray(
        xT_perm.reshape(32, 128, M_TOK).transpose(1, 0, 2)
        .reshape(128, 32 * M_TOK)).astype(np.float16)

    # group sums: S[g,m] = sum_{k in g} x'[k,m]   (zeros row-set)
    # U[g,m] = 16*sum_{pairs in g}(XA+XB) from the f16 tile values so the
    # spurious-16s cancellation is not limited by host-f32 rounding.
    xg = x2.T.reshape(32, 128, M_TOK)
    xf = xT_perm.astype(np.float16).astype(np.float32)    # fl16(perm'd x)
    xfg = xf.reshape(16, 2, 128, M_TOK)
    U = 16.0 * (xfg[:, 0] + xfg[:, 1])                    # [16, 128, M] pairs
    U = U.reshape(16, 2, 64, M_TOK).sum(axis=2).reshape(32, M_TOK)
    Uh = U.astype(np.float16)
    s16 = np.empty((97, M_TOK), np.float16)
    s16[0:32] = xg.sum(axis=1).astype(np.float16)
    s16[32:64] = Uh
    s16[64:96] = (U - Uh.astype(np.float32)).astype(np.float16)
    s16[96] = 1.0

    # srep: byte-row r of block b -> group 2b (rows 0-63) / 2b+1 (rows 64-127)
    scalesT = np.asarray(scales, np.float32).T            # [32, N]
    srep = np.empty((NB * 128, OUT_F), np.float16)
    for b in range(NB):
        srep[b * 128:b * 128 + 64] = scalesT[2 * b].astype(np.float16)
        srep[b * 128 + 64:(b + 1) * 128] = scalesT[2 * b + 1].astype(np.float16)

    zbT = np.empty((97, OUT_F), np.float16)
    zbT[0:32] = np.asarray(zeros, np.float32).T.astype(np.float16)
    zbT[32:64] = (-np.asarray(scales, np.float32).T).astype(np.float16)
    zbT[64:96] = zbT[32:64]
    zbT[96] = np.asarray(bias, np.float32).astype(np.float16)

    return qwT, srep, xT_b, s16, zbT


def make_in_maps(x, qweight, scales, zeros, channel_scales, bias):
    qwT, srep, xT_b, s16, zbT = _host_prep(
        x, qweight, scales, zeros, channel_scales, bias)
    xT_u8 = xT_b.view(np.uint8)                           # [128, 16384]
    in_maps = []
    for c in range(NCORES):
        sl = slice(c * NSH, (c + 1) * NSH)
        blkb = np.empty((NB * 128, BLKW), np.uint8)
        blkb[:, 0:NSH] = qwT[:, sl]
        blkb[:, NSH:] = srep[:, sl].copy().view(np.uint8).reshape(
            NB * 128, 2 * NSH)
        # fused pair rows: [blk(2p) | blk(2p+1) | x-pair] per 128-row group
        pairs = blkb.reshape(NP, 2, 128, BLKW).transpose(0, 2, 1, 3)
        blk = np.empty((NP, 128, PRW), np.uint8)
        blk[:, :, 0:2 * BLKW] = pairs.reshape(NP, 128, 2 * BLKW)
        blk[:, :, 2 * BLKW:] = xT_u8.reshape(128, NP, XW).transpose(1, 0, 2)
        in_maps.append({
            "blk": np.ascontiguousarray(blk.reshape(NP * 128, PRW)),
            "s16": s16,
            "zbT": np.ascontiguousarray(zbT[:, sl]),
        })
    return in_maps


_NC_CACHE = {}


def get_nc():
    if "nc" not in _NC_CACHE:
        _NC_CACHE["nc"] = _build_nc()
    return _NC_CACHE["nc"]


def kernel(x, qweight, scales, zeros, channel_scales, bias):
    in_maps = make_in_maps(x, qweight, scales, zeros, channel_scales, bias)
    nc = get_nc()
    res = run_bass_kernel_spmd(nc, in_maps, core_ids=list(range(NCORES)))
    y = np.concatenate([res.results[c]["y"] for c in range(NCORES)], axis=1)
    return y.reshape(1, M_TOK, OUT_F).astype(np.float32)


# revision 15
# speedup vs baseline: 1.0056x; 1.0056x over previous
"""AWQ W4A16-style quantized linear (nn_AWQLinear) on 8 Trainium2 NeuronCores.

y[m,n] = sum_k x[m,k] * ((wq[n,k]*scales[n,g(k)] + zeros[n,g(k)]) / cs[k]) + bias[n]

Column-parallel over out_features (8 cores, N_shard = 1376/core).

Fixed-point dequant via f16 exponent pinning (no 1x-rate DVE ops):

  t    = qb/16 + 16.0          ACT (free affine).  t in [16,32) -> f16
                               exponent constant, mantissa = 64h+4l is a
                               fixed-point field (exact).
  t_lo = t & 0x4C3C            DVE TS bitvec 2x  (= 16 + l/16, exact)
  wL   = t_lo * srep           DVE TT f16 2x     (= (16+l/16)*s)
  wD   = t    * srep           DVE TT f16 2x     (= (16+h+l/16)*s)
  y   += XA^T wL + XB^T wD     TensorE; XA = 16*x'_e - x'_o, XB = x'_o
                               (both host-baked into the x tiles)

XA*wL + XB*wD = s*(x_e*l + x_o*h) + 256*x_e*s.  The spurious term is
sum_g U[g,m]*s[g,n] with U[g,m] = 16*sum(XA+XB) over the group's f16
tile values -- cancelled exactly by the augmented tail matmul that also
applies zeros+bias:
  lhsT = [S | U_hi | U_lo | ones] (97 x 256), rhs = [zeros | -s | -s | bias].
Output stored f16 (budget 2e-2).

DMA: ONE fused row-contiguous ~1.3MB transfer per block-pair carrying
[qb|srep] for both blocks plus that pair's x tile (10304 B/row) -- the
whole kernel is ~17 DMAs on the nc.sync HWDGE ring, so the delivery
stream has no ordering hazards and descriptors are large.  GPSIMD stays
idle (SWDGE rings share SBUF ports with DVE).  A burst of dummy matmuls
at t=0 warms the PE HAM clock gate (1.2 -> 2.4 GHz).
"""
import numpy as np

import concourse.bacc as bacc
import concourse.mybir as mybir
from concourse import tile
from concourse.bass_utils import run_bass_kernel_spmd

IN_F = 4096          # K
OUT_F = 11008        # N
M_TOK = 256          # M
NCORES = 8
NSH = OUT_F // NCORES   # 1376
NB = IN_F // 256        # 16 blocks of 128 byte-rows (256 k-values each)
NP = NB // 2            # 8 pairs
CHUNKS = [(0, 512), (512, 512), (1024, NSH - 1024)]
BLKW = NSH + 2 * NSH    # 4128 bytes: [qb u8 | srep f16] per block row
XW = 4 * M_TOK * 2      # 2048 bytes of x per pair row (4 m-tiles f16)
PRW = 2 * BLKW + XW     # 10304 bytes per fused pair row

F32, F16, U8, U16 = (mybir.dt.float32, mybir.dt.float16, mybir.dt.uint8,
                     mybir.dt.uint16)


def _build_nc():
    nc = bacc.Bacc("TRN2", target_bir_lowering=False, debug=False,
                   num_devices=NCORES)

    blk_d = nc.dram_tensor("blk", [NP * 128, PRW], U8, kind="ExternalInput")
    s16_d = nc.dram_tensor("s16", [97, M_TOK], F16, kind="ExternalInput")
    zbT_d = nc.dram_tensor("zbT", [97, NSH], F16, kind="ExternalInput")
    y_d = nc.dram_tensor("y", [M_TOK, NSH], F16, kind="ExternalOutput")

    A = mybir.AluOpType
    W2 = 2 * NSH

    with tile.TileContext(nc) as tc:
        with (
            tc.tile_pool(name="const", bufs=1) as cpool,
            tc.tile_pool(name="blk", bufs=4) as blkpool,
            tc.tile_pool(name="t16", bufs=3) as tpool,
            tc.tile_pool(name="tl", bufs=3) as tlpool,
            tc.tile_pool(name="w", bufs=6) as wpool,
            tc.tile_pool(name="yout", bufs=3) as ypool,
            tc.tile_pool(name="ps", bufs=1, space="PSUM") as pspool,
        ):
            # ---- PE warmup: dummy matmuls to flip HAM to 8/8 early ----
            warm = cpool.tile([128, 640], F16)
            nc.vector.memset(warm[:], 0.0)
            warm_ps = pspool.tile([128, 512], F32, tag="warm_ps")
            for _ in range(8):
                nc.tensor.matmul(warm_ps[:], warm[:, 0:128], warm[:, 128:640],
                                 start=True, stop=True)

            # ---- psum accumulators ----
            y_ps = [[pspool.tile([128, w], F32, tag=f"yps_{m}_{ci}",
                                 name=f"yps_{m}_{ci}")
                     for ci, (_, w) in enumerate(CHUNKS)] for m in range(2)]

            state = {}

            def xtile(p, t):
                """m-tile t (0..3) of pair p's x: [XA_2p|XB_2p|XA_|XB_]."""
                xv = state[("blk", p)][:, 2 * BLKW:PRW].bitcast(F16)
                return xv[:, t * M_TOK:(t + 1) * M_TOK]

            def load_pair(p):
                blkp = blkpool.tile([128, PRW], U8, tag="blk",
                                    name=f"blk_{p}")
                src = blk_d[p * 128:(p + 1) * 128, :]
                if p >= 2:
                    # completion-chain token: cap pair DMAs in flight at 2.
                    # Reads pair p-2's tile (waits for its full delivery),
                    # writes one byte column of pair p's tile (WAW gates
                    # the big DMA below).  Single-queue round-robin would
                    # otherwise delay every completion to the end of the
                    # whole stream.
                    prev = state[("blk", p - 2)]
                    nc.sync.dma_start(blkp[:, PRW - 1:PRW],
                                      prev[:, PRW - 1:PRW])
                if p == 0:
                    # split for head latency; x rides part b
                    nc.sync.dma_start(blkp[:, 0:BLKW], src[:, 0:BLKW])
                    nc.sync.dma_start(blkp[:, BLKW:], src[:, BLKW:])
                else:
                    nc.sync.dma_start(blkp[:], src)
                state[("blk", p)] = blkp

            def dequant(p, j=None):
                """Dequant a pair (or one block j of pair 0)."""
                blkp = state[("blk", p)]
                pairv = blkp[:, 0:2 * BLKW].rearrange("p (j w) -> p j w", j=2)
                if j is None:
                    qb = pairv[:, :, 0:NSH]
                    srep = pairv[:, :, NSH:BLKW].bitcast(F16)
                    tt = tpool.tile([128, W2], F16, tag="t16", name=f"t16_{p}")
                    tl = tlpool.tile([128, W2], F16, tag="tl", name=f"tl_{p}")
                    wL = wpool.tile([128, W2], F16, tag="wL", name=f"wL_{p}")
                    wD = wpool.tile([128, W2], F16, tag="wD", name=f"wD_{p}")
                    tv = tt[:].rearrange("p (j n) -> p j n", j=2)
                    nc.scalar.activation(tv, qb,
                                         mybir.ActivationFunctionType.Copy,
                                         bias=16.0, scale=0.0625)
                    nc.vector.tensor_scalar(tl[:].bitcast(U16),
                                            in0=tt[:].bitcast(U16),
                                            scalar1=0x4C3C, scalar2=None,
                                            op0=A.bitwise_and)
                    nc.vector.tensor_tensor(
                        wL[:].rearrange("p (j n) -> p j n", j=2),
                        tl[:].rearrange("p (j n) -> p j n", j=2), srep, A.mult)
                    nc.vector.tensor_tensor(
                        wD[:].rearrange("p (j n) -> p j n", j=2),
                        tv, srep, A.mult)
                    state[p] = (wL, wD)
                    return
                qb = pairv[:, j, 0:NSH]
                srep = pairv[:, j, NSH:BLKW].bitcast(F16)
                if j == 0:
                    tt = tpool.tile([128, W2], F16, tag="t16", name=f"t16_{p}")
                    tl = tlpool.tile([128, W2], F16, tag="tl", name=f"tl_{p}")
                    wL = wpool.tile([128, W2], F16, tag="wL", name=f"wL_{p}")
                    wD = wpool.tile([128, W2], F16, tag="wD", name=f"wD_{p}")
                    state[("tiles", p)] = (tt, tl, wL, wD)
                else:
                    tt, tl, wL, wD = state[("tiles", p)]
                sl = slice(j * NSH, (j + 1) * NSH)
                nc.scalar.activation(tt[:, sl], qb,
                                     mybir.ActivationFunctionType.Copy,
                                     bias=16.0, scale=0.0625)
                nc.vector.tensor_scalar(tl[:, sl].bitcast(U16),
                                        in0=tt[:, sl].bitcast(U16),
                                        scalar1=0x4C3C, scalar2=None,
                                        op0=A.bitwise_and)
                nc.vector.tensor_tensor(wL[:, sl], tl[:, sl], srep, A.mult)
                nc.vector.tensor_tensor(wD[:, sl], tt[:, sl], srep, A.mult)
                if j == 1:
                    state[p] = (wL, wD)

            def mms(b):
                p, j = b // 2, b % 2
                wL, wD = state[p]
                XA = xtile(p, 2 * j)      # 16*x'_e - x'_o (host-baked)
                XB = xtile(p, 2 * j + 1)  # x'_o
                if b % 2 == 1:
                    state.pop(p)
                    state.pop(("tiles", p), None)
                sl = slice(j * NSH, (j + 1) * NSH)
                wLs, wDs = wL[:, sl], wD[:, sl]
                if b < NB - 2:
                    for w, xt in ((wLs, XA), (wDs, XB)):
                        for m in range(2):
                            for ci, (c0, cw) in enumerate(CHUNKS):
                                nc.tensor.matmul(
                                    y_ps[m][ci][:],
                                    xt[:, m * 128:(m + 1) * 128],
                                    w[:, c0:c0 + cw],
                                    start=(b == 0 and w is wLs), stop=False)
                    return
                if b == NB - 2:
                    return  # emitted together with b == NB-1 below
                # last pair: chunk-major over blocks 14+15 so each chunk's
                # correction-MM + copy + store overlap later chunks' matmuls
                for ci, (c0, cw) in enumerate(CHUNKS):
                    for jj in range(2):
                        bsl = slice(jj * NSH + c0, jj * NSH + c0 + cw)
                        XAj = xtile(p, 2 * jj)
                        XBj = xtile(p, 2 * jj + 1)
                        for w, xt in ((wL, XAj), (wD, XBj)):
                            for m in range(2):
                                nc.tensor.matmul(
                                    y_ps[m][ci][:],
                                    xt[:, m * 128:(m + 1) * 128],
                                    w[:, bsl],
                                    start=False, stop=False)
                    for m in range(2):
                        nc.tensor.matmul(y_ps[m][ci][:],
                                         s16[:, m * 128:(m + 1) * 128],
                                         zbT[:, c0:c0 + cw],
                                         start=False, stop=True)
                        ysb = ypool.tile([128, cw], F16, tag=f"ysb_{ci}",
                                         name=f"ysb_{m}_{ci}")
                        if (m + ci) % 2:
                            nc.scalar.copy(ysb[:], y_ps[m][ci][:])
                        else:
                            nc.vector.tensor_copy(ysb[:], y_ps[m][ci][:])
                        nc.sync.dma_start(
                            y_d[m * 128:(m + 1) * 128, c0:c0 + cw], ysb[:])

            # ---- software-pipelined main loop (pair granularity) ----
            # JIT DMA issue: at most ~2 pair-loads in flight, so the
            # single-queue round-robin cannot delay the oldest pair's
            # completion behind younger transfers.
            load_pair(0)
            dequant(0, j=0)
            load_pair(1)
            # tail-only constants: tiny, issue while the ring is quiet
            s16 = cpool.tile([97, M_TOK], F16)
            nc.sync.dma_start(s16[:], s16_d[:])
            zbT = cpool.tile([97, NSH], F16)
            nc.sync.dma_start(zbT[:], zbT_d[:])
            dequant(0, j=1)
            for p in range(1, NP + 1):
                if p < NP:
                    if p + 1 < NP:
                        load_pair(p + 1)
                    dequant(p)
                mms(2 * (p - 1))
                mms(2 * (p - 1) + 1)

    nc.compile()
    return nc


def _host_prep(x, qweight, scales, zeros, channel_scales, bias):
    cs = np.asarray(channel_scales, np.float32)
    x2 = np.asarray(x, dtype=np.float32).reshape(M_TOK, IN_F) / cs
    qw = np.asarray(qweight)
    if qw.dtype != np.uint8:
        qw = qw.astype(np.uint8)
    qwT = np.ascontiguousarray(qw.T)                      # [K/2, N]

    # x tiles: block b byte-row q holds k_even=256b+2q, k_odd=256b+2q+1.
    # XA-slot (even) = 16*x'_e - x'_o, XB-slot (odd) = x'_o  (host-baked)
    xe = x2[:, 0::2]
    xo = x2[:, 1::2]
    xs = np.empty_like(x2)
    xs[:, 0::2] = 16.0 * xe - xo
    xs[:, 1::2] = xo
    q = np.arange(128)
    perm = np.empty(IN_F, np.int64)
    for b in range(NB):
        perm[(2 * b) * 128 + q] = 256 * b + 2 * q
        perm[(2 * b + 1) * 128 + q] = 256 * b + 2 * q + 1
    xT_perm = xs.T[perm]                                  # [K, M]
    xT_b = np.ascontiguousarray(
        xT_perm.reshape(32, 128, M_TOK).transpose(1, 0, 2)
        .reshape(128, 32 * M_TOK)).astype(np.float16)

    # group sums: S[g,m] = sum_{k in g} x'[k,m]   (zeros row-set)
    # U[g,m] = 16*sum_{pairs in g}(XA+XB) from the f16 tile values so the
    # spurious-16s cancellation is not limited by host-f32 rounding.
    xg = x2.T.reshape(32, 128, M_TOK)
    xf = xT_perm.astype(np.float16).astype(np.float32)    # fl16(perm'd x)
    xfg = xf.reshape(16, 2, 128, M_TOK)
    U = 16.0 * (xfg[:, 0] + xfg[:, 1])                    # [16, 128, M] pairs
    U = U.reshape(16, 2, 64, M_TOK).sum(axis=2).reshape(32, M_TOK)
    Uh = U.astype(np.float16)
    s16 = np.empty((97, M_TOK), np.float16)
    s16[0:32] = xg.sum(axis=1).astype(np.float16)
    s16[32:64] = Uh
    s16[64:96] = (U - Uh.astype(np.float32)).astype(np.float16)
    s16[96] = 1.0

    # srep: byte-row r of block b -> group 2b (rows 0-63) / 2b+1 (rows 64-127)
    scalesT = np.asarray(scales, np.float32).T            # [32, N]
    srep = np.empty((NB * 128, OUT_F), np.float16)
    for b in range(NB):
        srep[b * 128:b * 128 + 64] = scalesT[2 * b].astype(np.float16)
        srep[b * 128 + 64:(b + 1) * 128] = scalesT[2 * b + 1].astype(np.float16)

    zbT = np.empty((97, OUT_F), np.float16)
    zbT[0:32] = np.asarray(zeros, np.float32).T.astype(np.float16)
    zbT[32:64] = (-np.asarray(scales, np.float32).T).astype(np.float16)
    zbT[64:96] = zbT[32:64]
    zbT[96] = np.asarray(bias, np.float32).astype(np.float16)

    return qwT, srep, xT_b, s16, zbT


def make_in_maps(x, qweight, scales, zeros, channel_scales, bias):
    qwT, srep, xT_b, s16, zbT = _host_prep(
        x, qweight, scales, zeros, channel_scales, bias)
    xT_u8 = xT_b.view(np.uint8)                           # [128, 16384]
    in_maps = []
    for c in range(NCORES):
        sl = slice(c * NSH, (c + 1) * NSH)
        blkb = np.empty((NB * 128, BLKW), np.uint8)
        blkb[:, 0:NSH] = qwT[:, sl]
        blkb[:, NSH:] = srep[:, sl].copy().view(np.uint8).reshape(
            NB * 128, 2 * NSH)
        # fused pair rows: [blk(2p) | blk(2p+1) | x-pair] per 128-row group
        pairs = blkb.reshape(NP, 2, 128, BLKW).transpose(0, 2, 1, 3)
        blk = np.empty((NP, 128, PRW), np.uint8)
        blk[:, :, 0:2 * BLKW] = pairs.reshape(NP, 128, 2 * BLKW)
        blk[:, :, 2 * BLKW:] = xT_u8.reshape(128, NP, XW).transpose(1, 0, 2)
        in_maps.append({
            "blk": np.ascontiguousarray(blk.reshape(NP * 128, PRW)),
            "s16": s16,
            "zbT": np.ascontiguousarray(zbT[:, sl]),
        })
    return in_maps


_NC_CACHE = {}


def get_nc():
    if "nc" not in _NC_CACHE:
        _NC_CACHE["nc"] = _build_nc()
    return _NC_CACHE["nc"]


def kernel(x, qweight, scales, zeros, channel_scales, bias):
    in_maps = make_in_maps(x, qweight, scales, zeros, channel_scales, bias)
    nc = get_nc()
    res = run_bass_kernel_spmd(nc, in_maps, core_ids=list(range(NCORES)))
    y = np.concatenate([res.results[c]["y"] for c in range(NCORES)], axis=1)
    return y.reshape(1, M_TOK, OUT_F).astype(np.float32)
